# revision 47
# baseline (speedup 1.0000x reference)
"""CLIP loss (nn_CLIPLoss) on 8 Trainium2 NeuronCores.

loss = 0.5*(mean_i LSE_row(i) + mean_j LSE_col(j)) - mean_i <x_i, y_i>
with logits = x @ y.T, N=16384, D=256.

Strategy: shard x row-wise across 8 cores (2048 rows each); every core holds
all of y (host pre-transposes/casts both to fp16 [D, *] so the contraction dim
sits on SBUF partitions). Core k computes its 2048x16384 block of logits
tile-by-tile on the TensorEngine (fp16 operands, fp32 PSUM accumulate, 2048-
wide double-buffered PSUM tiles), applies exp(logit - C) on the ScalarEngine
(bf16 output for range, fp32 accum_out producing row-sums for free), and
accumulates the E tiles on VectorE (bf16 2x mode); the 128-partition column
fold is 4 col-group-concurrent ones-matmuls on the TensorEngine into an
already-drained logits PSUM tile. The diagonal <x_i, y_i> is an elementwise
product + ones-matmul on the same fp16 operands. The host sums the per-core
partials in float64, takes logs, and combines into the scalar.

The full NxN logits matrix is never materialized anywhere. Measured ~312 us
on hardware per 8-core run; ScalarE exp throughput (~266 us) is the floor.
"""

import os
import sys
import types

import numpy as np

N = 16384
D = 256
NCORES = 8
S = N // NCORES  # 2048 rows per core
P = 128
C_OFF = 64.0  # exp offset: logits in ~[-100, 100]; exp(l - 64) stays in fp32/bf16 range
NT = 8  # 2048-wide column chunks
CHUNK = 2048
MT = S // P  # 16 row tiles per core
YT_PIECES = 8  # split the big y-transpose load into 8 DMAs for pipelining


def _ensure_axon_hooks():
    """The agent image's antenv lacks axon_hooks; recreate it and register the
    ctypes NTFF profile hook so BASS_TRACE=1 yields exec_time_ns. Harmless
    no-op when profiling infra is absent."""
    try:
        import antenv.axon_hooks  # noqa: F401

        return
    except ImportError:
        pass
    try:
        import antenv
    except ImportError:
        return

    mod = types.ModuleType("antenv.axon_hooks")
    mod._hook = None

    def set_axon_ntff_profile_hook(h):
        mod._hook = h

    def get_axon_ntff_profile_hook():
        return mod._hook

    mod.set_axon_ntff_profile_hook = set_axon_ntff_profile_hook
    mod.get_axon_ntff_profile_hook = get_axon_ntff_profile_hook
    sys.modules["antenv.axon_hooks"] = mod
    antenv.axon_hooks = mod
    try:
        from trn_agent_boot.trn_boot import _ntff_profile_via_ctypes

        hook = _ntff_profile_via_ctypes("/opt/axon/libaxon_pjrt.so")
        if hook is not None:
            mod._hook = hook
    except Exception:
        pass


_ensure_axon_hooks()

_NC_CACHE = None
LAST_RESULTS = None  # test harness reads .exec_time_ns off this


def _build_nc():
    import concourse.mybir as mybir
    import concourse.tile as tile
    from concourse import bacc

    f16 = mybir.dt.float16
    bf16 = mybir.dt.bfloat16
    f32 = mybir.dt.float32
    Exp = mybir.ActivationFunctionType.Exp

    nc = bacc.Bacc("TRN2", target_bir_lowering=False, debug=False, num_devices=NCORES)

    xt = nc.dram_tensor("xt", [D, S], f16, kind="ExternalInput").ap()
    yt = nc.dram_tensor("yt", [D, N], f16, kind="ExternalInput").ap()
    ytk = nc.dram_tensor("ytk", [D, S], f16, kind="ExternalInput").ap()
    out_rows = nc.dram_tensor("rowsums", [P, MT * NT], f32, kind="ExternalOutput").ap()
    out_cols = nc.dram_tensor("colsums", [4, NT * 512], f32, kind="ExternalOutput").ap()
    out_diag = nc.dram_tensor("diag", [4, 512], f32, kind="ExternalOutput").ap()

    # d-major layouts with the 256-long d axis split as (chunk c, partition p)
    xt_r = xt.rearrange("(c p) s -> p c s", p=P)
    yt_r = yt.rearrange("(c p) n -> p c n", p=P)
    ytk_r = ytk.rearrange("(c p) s -> p c s", p=P)

    with tile.TileContext(nc) as tc:
        with (
            tc.tile_pool(name="singles", bufs=1) as singles,
            tc.tile_pool(name="epool", bufs=3) as epool,
        ):
            # persistent SBUF residents
            xt_sb = singles.tile([P, 2, S], f16, tag="xt")
            ytk_sb = singles.tile([P, 2, S], f16, tag="ytk")
            # y-transpose pieces; the first two are small so the first
            # matmuls can start as early as possible
            piece_widths = [512, 1536] + [CHUNK] * (NT - 1)
            piece_starts = [0]
            for w in piece_widths:
                piece_starts.append(piece_starts[-1] + w)
            assert piece_starts[-1] == N
            yt_sb = [
                singles.tile([P, 2, w], f16, tag=f"yt{j}", name=f"yt{j}")
                for j, w in enumerate(piece_widths)
            ]

            def yt_slice(c, g0, width):
                """AP into the yt piece covering global cols [g0, g0+width)."""
                for j, s in enumerate(piece_starts[:-1]):
                    if s <= g0 and g0 + width <= piece_starts[j + 1]:
                        return yt_sb[j][:, c, g0 - s : g0 - s + width]
                raise AssertionError(f"no piece covers [{g0}, {g0 + width})")
            ones_f16 = singles.tile([P, 1], f16, tag="ones_f16")
            bias_sb = singles.tile([P, 1], f32, tag="bias")
            rowsum_all = singles.tile([P, MT * NT], f32, tag="rowsum")
            # colsum lives on partitions {0,32,64,96}: row 32q, col nt*512+r
            # holds colsum[nt*2048 + q*512 + r]
            colsum_sb = singles.tile([P, NT * 512], f32, tag="colsum")
            diag_sb = singles.tile([P, 512], f32, tag="diag")
            prod_sb = singles.tile([P, 2, S], f16, tag="prod")

            nc.vector.memset(ones_f16, 1.0)
            nc.vector.memset(bias_sb, -C_OFF)

            # Two HWDGE rings in parallel: yt pieces stream on the sync ring
            # in chunk order; xt halves + ytk (diag-only, consumed last) go on
            # the scalar ring so the first matmul group's inputs land fastest.
            for j, w in enumerate(piece_widths):
                s = piece_starts[j]
                nc.sync.dma_start(out=yt_sb[j], in_=yt_r[:, :, s : s + w])
            nc.scalar.dma_start(out=xt_sb[:, 0, :], in_=xt_r[:, 0, :])
            nc.scalar.dma_start(out=xt_sb[:, 1, :], in_=xt_r[:, 1, :])
            nc.scalar.dma_start(out=ytk_sb, in_=ytk_r)
            # elementwise x*y for the diagonal; runs early on the idle DVE,
            # consumed by the ones-matmuls after the main loop frees PSUM
            nc.vector.tensor_mul(prod_sb, xt_sb, ytk_sb)

            # ---- main loop over the 2048 x 16384 logits block ----
            # Per 2048-wide chunk: 16 row-tiles of matmul (fp16, 4x 512-wide
            # PSUM sub-tiles) + one wide exp on ScalarE (accum_out = row sums);
            # DVE accumulates the 16 bf16 E tiles (2x mode) then folds the
            # 128 partitions with a halving tree. PSUM: 2x 4-bank logits tiles.
            with (
                tc.tile_pool(name="pslogits", bufs=2, space="PSUM") as ps_pool,
                tc.tile_pool(name="accpool", bufs=2) as accpool,
                tc.tile_pool(name="onespool", bufs=1) as onespool,
            ):
                ones_acc = onespool.tile([P, 1], bf16, tag="ones_acc")
                nc.vector.memset(ones_acc, 1.0)

                def emit_colsum(pend, target_ps):
                    # Partition fold of the finished acc via 4 ones-matmuls
                    # into PE col-groups 0/32/64/96 (concurrent on the
                    # sub-arrays), landing in an already-drained logits PSUM
                    # tile; one wide [97, 512] copy (partitions 1..96 carry
                    # don't-care data) moves rows {0,32,64,96} to SBUF.
                    p_acc, p_nt = pend
                    for q in range(CHUNK // 512):
                        o = q * 512
                        nc.tensor.matmul(
                            target_ps[32 * q : 32 * q + 1, 0:512],
                            ones_acc,
                            p_acc[:, o : o + 512],
                            start=True,
                            stop=True,
                            tile_position=(0, 32 * q),
                        )
                    nc.vector.tensor_copy(
                        colsum_sb[0:97, p_nt * 512 : (p_nt + 1) * 512],
                        target_ps[0:97, 0:512],
                    )
                    # stream this chunk's finished colsum out now instead of
                    # piling all output DMAs into the kernel tail
                    nc.sync.dma_start(
                        out=out_cols[:, p_nt * 512 : (p_nt + 1) * 512],
                        in_=colsum_sb[0:97:32, p_nt * 512 : (p_nt + 1) * 512],
                    )

                pending = None  # colsum fold lags three tiles into next chunk
                for nt in range(NT):
                    acc = accpool.tile([P, CHUNK], bf16, tag="acc")
                    chunk_ps1 = None
                    for mt in range(MT):
                        ps = ps_pool.tile([P, CHUNK], f32)
                        if mt == 1:
                            chunk_ps1 = ps
                        for c in range(2):
                            for q in range(CHUNK // 512):
                                o = q * 512
                                nc.tensor.matmul(
                                    ps[:, o : o + 512],
                                    xt_sb[:, c, mt * P : (mt + 1) * P],
                                    yt_slice(c, nt * CHUNK + o, 512),
                                    start=(c == 0),
                                    stop=(c == 1),
                                )
                        if pending is not None and mt == 2:
                            emit_colsum(pending, chunk_ps1)
                            pending = None
                        e_sb = epool.tile([P, CHUNK], bf16, tag="e")
                        idx = mt * NT + nt
                        nc.scalar.activation(
                            out=e_sb,
                            in_=ps,
                            func=Exp,
                            bias=bias_sb[:],
                            scale=1.0,
                            accum_out=rowsum_all[:, idx : idx + 1],
                        )
                        if mt == 0:
                            nc.vector.tensor_copy(acc, e_sb)
                        else:
                            nc.vector.tensor_add(acc, acc, e_sb)
                    pending = (acc, nt)
                # last chunk: fold into its own final tile (ps still in scope)
                emit_colsum(pending, ps)

            # ---- diagonal: diag[s] = sum_d x[s,d]*y[s,d] (PSUM now free) ----
            # col-grouped ones-matmuls (concurrent) into one PSUM bank; rows
            # {0,32,64,96} x 512 hold diag[q*512 + r]
            with tc.tile_pool(name="psdiag", bufs=1, space="PSUM") as dg_pool:
                diag_ps = dg_pool.tile([P, 512], f32)
                for c in range(2):
                    for q in range(S // 512):
                        nc.tensor.matmul(
                            diag_ps[32 * q : 32 * q + 1, 0:512],
                            ones_f16,
                            prod_sb[:, c, q * 512 : (q + 1) * 512],
                            start=(c == 0),
                            stop=(c == 1),
                            tile_position=(0, 32 * q),
                        )
                nc.vector.tensor_copy(diag_sb[0:97, :], diag_ps[0:97, :])
            nc.sync.dma_start(out=out_diag, in_=diag_sb[0:97:32, :])

            nc.sync.dma_start(out=out_rows, in_=rowsum_all)

    nc.compile()
    return nc


def kernel(x, y):
    global _NC_CACHE, LAST_RESULTS
    from concourse.bass_utils import run_bass_kernel_spmd

    x = np.asarray(x, dtype=np.float32)
    y = np.asarray(y, dtype=np.float32)
    assert x.shape == (N, D) and y.shape == (N, D)

    xt_all = np.ascontiguousarray(x.T.astype(np.float16))  # [D, N]
    yt_all = np.ascontiguousarray(y.T.astype(np.float16))  # [D, N]

    in_maps = []
    for k in range(NCORES):
        sl = slice(k * S, (k + 1) * S)
        in_maps.append(
            {
                "xt": np.ascontiguousarray(xt_all[:, sl]),
                "yt": yt_all,
                "ytk": np.ascontiguousarray(yt_all[:, sl]),
            }
        )

    if _NC_CACHE is None:
        _NC_CACHE = _build_nc()
    nc = _NC_CACHE

    res = run_bass_kernel_spmd(nc, in_maps, core_ids=list(range(NCORES)))
    LAST_RESULTS = res

    rowsum = np.empty(N, dtype=np.float64)
    colsum = np.zeros(N, dtype=np.float64)
    diag = np.empty(N, dtype=np.float64)
    for k in range(NCORES):
        r = res.results[k]
        # rowsums [P, MT*NT]: partition p, col mt*NT + nt -> row k*S + mt*P + p
        rs = r["rowsums"].astype(np.float64).reshape(P, MT, NT).sum(-1)  # [P, MT]
        rowsum[k * S : (k + 1) * S] = rs.T.reshape(S)
        # colsums [4, NT*512]: row q, col nt*512+r -> colsum[nt*2048+q*512+r]
        cs = r["colsums"].astype(np.float64).reshape(4, NT, 512)
        colsum += np.transpose(cs, (1, 0, 2)).reshape(N)
        diag[k * S : (k + 1) * S] = r["diag"].astype(np.float64).reshape(S)

    lse_row = C_OFF + np.log(rowsum)
    lse_col = C_OFF + np.log(colsum)
    loss = 0.5 * (lse_row.mean() + lse_col.mean()) - diag.mean()
    return np.float32(loss)


# revision 49
# speedup vs baseline: 1.0267x; 1.0267x over previous
"""CLIP loss (nn_CLIPLoss) on 8 Trainium2 NeuronCores.

loss = 0.5*(mean_i LSE_row(i) + mean_j LSE_col(j)) - mean_i <x_i, y_i>
with logits = x @ y.T, N=16384, D=256.

Strategy: shard x row-wise across 8 cores (2048 rows each); every core holds
all of y (host pre-transposes/casts both to fp16 [D, *] so the contraction dim
sits on SBUF partitions). Core k computes its 2048x16384 block of logits
tile-by-tile on the TensorEngine (fp16 operands, fp32 PSUM accumulate, 2048-
wide double-buffered PSUM tiles), applies exp(logit - C) on the ScalarEngine
(bf16 output for range, fp32 accum_out producing row-sums for free), and
accumulates the E tiles on VectorE (bf16 2x mode); the 128-partition column
fold is 4 col-group-concurrent ones-matmuls on the TensorEngine into an
already-drained logits PSUM tile. The diagonal <x_i, y_i> is an elementwise
product + ones-matmul on the same fp16 operands. The host sums the per-core
partials in float64, takes logs, and combines into the scalar.

The full NxN logits matrix is never materialized anywhere. Measured ~312 us
on hardware per 8-core run; ScalarE exp throughput (~266 us) is the floor.
"""

import os
import sys
import types

import numpy as np

N = 16384
D = 256
NCORES = 8
S = N // NCORES  # 2048 rows per core
P = 128
C_OFF = 64.0  # exp offset: logits in ~[-100, 100]; exp(l - 64) stays in fp32/bf16 range
NT = 8  # 2048-wide column chunks
CHUNK = 2048
MT = S // P  # 16 row tiles per core
YT_PIECES = 8  # split the big y-transpose load into 8 DMAs for pipelining


def _ensure_axon_hooks():
    """The agent image's antenv lacks axon_hooks; recreate it and register the
    ctypes NTFF profile hook so BASS_TRACE=1 yields exec_time_ns. Harmless
    no-op when profiling infra is absent."""
    try:
        import antenv.axon_hooks  # noqa: F401

        return
    except ImportError:
        pass
    try:
        import antenv
    except ImportError:
        return

    mod = types.ModuleType("antenv.axon_hooks")
    mod._hook = None

    def set_axon_ntff_profile_hook(h):
        mod._hook = h

    def get_axon_ntff_profile_hook():
        return mod._hook

    mod.set_axon_ntff_profile_hook = set_axon_ntff_profile_hook
    mod.get_axon_ntff_profile_hook = get_axon_ntff_profile_hook
    sys.modules["antenv.axon_hooks"] = mod
    antenv.axon_hooks = mod
    try:
        from trn_agent_boot.trn_boot import _ntff_profile_via_ctypes

        hook = _ntff_profile_via_ctypes("/opt/axon/libaxon_pjrt.so")
        if hook is not None:
            mod._hook = hook
    except Exception:
        pass


_ensure_axon_hooks()

_NC_CACHE = None
LAST_RESULTS = None  # test harness reads .exec_time_ns off this


def _build_nc():
    import concourse.mybir as mybir
    import concourse.tile as tile
    from concourse import bacc

    f16 = mybir.dt.float16
    bf16 = mybir.dt.bfloat16
    f32 = mybir.dt.float32
    Exp = mybir.ActivationFunctionType.Exp

    nc = bacc.Bacc("TRN2", target_bir_lowering=False, debug=False, num_devices=NCORES)

    xt = nc.dram_tensor("xt", [D, S], f16, kind="ExternalInput").ap()
    yt = nc.dram_tensor("yt", [D, N], f16, kind="ExternalInput").ap()
    ytk = nc.dram_tensor("ytk", [D, S], f16, kind="ExternalInput").ap()
    out_rows = nc.dram_tensor("rowsums", [P, MT * NT], f32, kind="ExternalOutput").ap()
    out_cols = nc.dram_tensor("colsums", [4, NT * 512], f32, kind="ExternalOutput").ap()
    out_diag = nc.dram_tensor("diag", [4, 512], f32, kind="ExternalOutput").ap()

    # d-major layouts with the 256-long d axis split as (chunk c, partition p)
    xt_r = xt.rearrange("(c p) s -> p c s", p=P)
    yt_r = yt.rearrange("(c p) n -> p c n", p=P)
    ytk_r = ytk.rearrange("(c p) s -> p c s", p=P)

    with tile.TileContext(nc) as tc:
        with (
            tc.tile_pool(name="singles", bufs=1) as singles,
            tc.tile_pool(name="epool", bufs=3) as epool,
        ):
            # persistent SBUF residents
            xt_sb = singles.tile([P, 2, S], f16, tag="xt")
            ytk_sb = singles.tile([P, 2, S], f16, tag="ytk")
            # y-transpose pieces; the first two are small so the first
            # matmuls can start as early as possible
            piece_widths = [512, 1536] + [CHUNK] * (NT - 1)
            piece_starts = [0]
            for w in piece_widths:
                piece_starts.append(piece_starts[-1] + w)
            assert piece_starts[-1] == N
            yt_sb = [
                singles.tile([P, 2, w], f16, tag=f"yt{j}", name=f"yt{j}")
                for j, w in enumerate(piece_widths)
            ]

            def yt_slice(c, g0, width):
                """AP into the yt piece covering global cols [g0, g0+width)."""
                for j, s in enumerate(piece_starts[:-1]):
                    if s <= g0 and g0 + width <= piece_starts[j + 1]:
                        return yt_sb[j][:, c, g0 - s : g0 - s + width]
                raise AssertionError(f"no piece covers [{g0}, {g0 + width})")
            ones_f16 = singles.tile([P, 1], f16, tag="ones_f16")
            bias_sb = singles.tile([P, 1], f32, tag="bias")
            rowsum_all = singles.tile([P, MT * NT], f32, tag="rowsum")
            # colsum lives on partitions {0,32,64,96}: row 32q, col nt*512+r
            # holds colsum[nt*2048 + q*512 + r]
            colsum_sb = singles.tile([P, NT * 512], f32, tag="colsum")
            diag_sb = singles.tile([P, 512], f32, tag="diag")
            prod_sb = singles.tile([P, 2, S], f16, tag="prod")

            nc.vector.memset(ones_f16, 1.0)
            nc.vector.memset(bias_sb, -C_OFF)

            # One sync ring (the ACT-issued ring starts too slowly), ordered
            # by first use: main(0,0) needs yt piece 0 + both xt halves; ytk
            # (diag-only) is consumed last.
            nc.sync.dma_start(out=yt_sb[0], in_=yt_r[:, :, 0 : piece_widths[0]])
            nc.sync.dma_start(out=xt_sb[:, 0, :], in_=xt_r[:, 0, :])
            nc.sync.dma_start(out=xt_sb[:, 1, :], in_=xt_r[:, 1, :])
            for j in range(1, len(piece_widths)):
                s = piece_starts[j]
                nc.sync.dma_start(
                    out=yt_sb[j], in_=yt_r[:, :, s : s + piece_widths[j]]
                )
            nc.sync.dma_start(out=ytk_sb, in_=ytk_r)
            # elementwise x*y for the diagonal; runs early on the idle DVE,
            # consumed by the ones-matmuls after the main loop frees PSUM
            nc.vector.tensor_mul(prod_sb, xt_sb, ytk_sb)

            # ---- main loop over the 2048 x 16384 logits block ----
            # Per 2048-wide chunk: 16 row-tiles of matmul (fp16, 4x 512-wide
            # PSUM sub-tiles) + one wide exp on ScalarE (accum_out = row sums);
            # DVE accumulates the 16 bf16 E tiles (2x mode) then folds the
            # 128 partitions with a halving tree. PSUM: 2x 4-bank logits tiles.
            with (
                tc.tile_pool(name="pslogits", bufs=2, space="PSUM") as ps_pool,
                tc.tile_pool(name="accpool", bufs=2) as accpool,
                tc.tile_pool(name="onespool", bufs=1) as onespool,
            ):
                ones_acc = onespool.tile([P, 1], bf16, tag="ones_acc")
                nc.vector.memset(ones_acc, 1.0)

                def emit_colsum(pend, target_ps):
                    # Partition fold of the finished acc via 4 ones-matmuls
                    # into PE col-groups 0/32/64/96 (concurrent on the
                    # sub-arrays), landing in an already-drained logits PSUM
                    # tile; one wide [97, 512] copy (partitions 1..96 carry
                    # don't-care data) moves rows {0,32,64,96} to SBUF.
                    p_acc, p_nt = pend
                    for q in range(CHUNK // 512):
                        o = q * 512
                        nc.tensor.matmul(
                            target_ps[32 * q : 32 * q + 1, 0:512],
                            ones_acc,
                            p_acc[:, o : o + 512],
                            start=True,
                            stop=True,
                            tile_position=(0, 32 * q),
                        )
                    nc.vector.tensor_copy(
                        colsum_sb[0:97, p_nt * 512 : (p_nt + 1) * 512],
                        target_ps[0:97, 0:512],
                    )
                    # stream this chunk's finished colsum out now instead of
                    # piling all output DMAs into the kernel tail
                    nc.sync.dma_start(
                        out=out_cols[:, p_nt * 512 : (p_nt + 1) * 512],
                        in_=colsum_sb[0:97:32, p_nt * 512 : (p_nt + 1) * 512],
                    )

                pending = None  # colsum fold lags three tiles into next chunk
                for nt in range(NT):
                    acc = accpool.tile([P, CHUNK], bf16, tag="acc")
                    chunk_ps1 = None
                    for mt in range(MT):
                        ps = ps_pool.tile([P, CHUNK], f32)
                        if mt == 1:
                            chunk_ps1 = ps
                        for c in range(2):
                            for q in range(CHUNK // 512):
                                o = q * 512
                                nc.tensor.matmul(
                                    ps[:, o : o + 512],
                                    xt_sb[:, c, mt * P : (mt + 1) * P],
                                    yt_slice(c, nt * CHUNK + o, 512),
                                    start=(c == 0),
                                    stop=(c == 1),
                                )
                        if pending is not None and mt == 2:
                            emit_colsum(pending, chunk_ps1)
                            pending = None
                        e_sb = epool.tile([P, CHUNK], bf16, tag="e")
                        idx = mt * NT + nt
                        nc.scalar.activation(
                            out=e_sb,
                            in_=ps,
                            func=Exp,
                            bias=bias_sb[:],
                            scale=1.0,
                            accum_out=rowsum_all[:, idx : idx + 1],
                        )
                        if mt == 0:
                            nc.vector.tensor_copy(acc, e_sb)
                        else:
                            nc.vector.tensor_add(acc, acc, e_sb)
                    pending = (acc, nt)
                # last chunk: fold into its own final tile (ps still in scope)
                emit_colsum(pending, ps)

            # ---- diagonal: diag[s] = sum_d x[s,d]*y[s,d] (PSUM now free) ----
            # col-grouped ones-matmuls (concurrent) into one PSUM bank; rows
            # {0,32,64,96} x 512 hold diag[q*512 + r]
            with tc.tile_pool(name="psdiag", bufs=1, space="PSUM") as dg_pool:
                diag_ps = dg_pool.tile([P, 512], f32)
                for c in range(2):
                    for q in range(S // 512):
                        nc.tensor.matmul(
                            diag_ps[32 * q : 32 * q + 1, 0:512],
                            ones_f16,
                            prod_sb[:, c, q * 512 : (q + 1) * 512],
                            start=(c == 0),
                            stop=(c == 1),
                            tile_position=(0, 32 * q),
                        )
                nc.vector.tensor_copy(diag_sb[0:97, :], diag_ps[0:97, :])
            nc.sync.dma_start(out=out_diag, in_=diag_sb[0:97:32, :])

            nc.sync.dma_start(out=out_rows, in_=rowsum_all)

    nc.compile()
    return nc


def kernel(x, y):
    global _NC_CACHE, LAST_RESULTS
    from concourse.bass_utils import run_bass_kernel_spmd

    x = np.asarray(x, dtype=np.float32)
    y = np.asarray(y, dtype=np.float32)
    assert x.shape == (N, D) and y.shape == (N, D)

    xt_all = np.ascontiguousarray(x.T.astype(np.float16))  # [D, N]
    yt_all = np.ascontiguousarray(y.T.astype(np.float16))  # [D, N]

    in_maps = []
    for k in range(NCORES):
        sl = slice(k * S, (k + 1) * S)
        in_maps.append(
            {
                "xt": np.ascontiguousarray(xt_all[:, sl]),
                "yt": yt_all,
                "ytk": np.ascontiguousarray(yt_all[:, sl]),
            }
        )

    if _NC_CACHE is None:
        _NC_CACHE = _build_nc()
    nc = _NC_CACHE

    try:
        res = run_bass_kernel_spmd(nc, in_maps, core_ids=list(range(NCORES)))
    except Exception:
        # The tunneled device occasionally reports unrecoverable and resets
        # itself within a minute; one retry saves the run.
        import time

        time.sleep(75)
        res = run_bass_kernel_spmd(nc, in_maps, core_ids=list(range(NCORES)))
    LAST_RESULTS = res

    rowsum = np.empty(N, dtype=np.float64)
    colsum = np.zeros(N, dtype=np.float64)
    diag = np.empty(N, dtype=np.float64)
    for k in range(NCORES):
        r = res.results[k]
        # rowsums [P, MT*NT]: partition p, col mt*NT + nt -> row k*S + mt*P + p
        rs = r["rowsums"].astype(np.float64).reshape(P, MT, NT).sum(-1)  # [P, MT]
        rowsum[k * S : (k + 1) * S] = rs.T.reshape(S)
        # colsums [4, NT*512]: row q, col nt*512+r -> colsum[nt*2048+q*512+r]
        cs = r["colsums"].astype(np.float64).reshape(4, NT, 512)
        colsum += np.transpose(cs, (1, 0, 2)).reshape(N)
        diag[k * S : (k + 1) * S] = r["diag"].astype(np.float64).reshape(S)

    lse_row = C_OFF + np.log(rowsum)
    lse_col = C_OFF + np.log(colsum)
    loss = 0.5 * (lse_row.mean() + lse_col.mean()) - diag.mean()
    return np.float32(loss)


# revision 52
# speedup vs baseline: 1.0566x; 1.0292x over previous
"""CLIP loss (nn_CLIPLoss) on 8 Trainium2 NeuronCores.

loss = 0.5*(mean_i LSE_row(i) + mean_j LSE_col(j)) - mean_i <x_i, y_i>
with logits = x @ y.T, N=16384, D=256.

Strategy: shard x row-wise across 8 cores (2048 rows each); every core holds
all of y (host pre-transposes/casts both to fp16 [D, *] so the contraction dim
sits on SBUF partitions). Core k computes its 2048x16384 block of logits
tile-by-tile on the TensorEngine (fp16 operands, fp32 PSUM accumulate, 2048-
wide double-buffered PSUM tiles), applies exp(logit - C) on the ScalarEngine
(bf16 output for range, fp32 accum_out producing row-sums for free), and
accumulates the E tiles on VectorE (bf16 2x mode); the 128-partition column
fold is 4 col-group-concurrent ones-matmuls on the TensorEngine into an
already-drained logits PSUM tile. The diagonal <x_i, y_i> is an elementwise
product + ones-matmul on the same fp16 operands. The host sums the per-core
partials in float64, takes logs, and combines into the scalar.

The full NxN logits matrix is never materialized anywhere. Measured ~312 us
on hardware per 8-core run; ScalarE exp throughput (~266 us) is the floor.
"""

import os
import sys
import types

import numpy as np

N = 16384
D = 256
NCORES = 8
S = N // NCORES  # 2048 rows per core
P = 128
C_OFF = 64.0  # exp offset: logits in ~[-100, 100]; exp(l - 64) stays in fp32/bf16 range
NT = 8  # 2048-wide column chunks
CHUNK = 2048
MT = S // P  # 16 row tiles per core
YT_PIECES = 8  # split the big y-transpose load into 8 DMAs for pipelining


def _ensure_axon_hooks():
    """The agent image's antenv lacks axon_hooks; recreate it and register the
    ctypes NTFF profile hook so BASS_TRACE=1 yields exec_time_ns. Harmless
    no-op when profiling infra is absent."""
    try:
        import antenv.axon_hooks  # noqa: F401

        return
    except ImportError:
        pass
    try:
        import antenv
    except ImportError:
        return

    mod = types.ModuleType("antenv.axon_hooks")
    mod._hook = None

    def set_axon_ntff_profile_hook(h):
        mod._hook = h

    def get_axon_ntff_profile_hook():
        return mod._hook

    mod.set_axon_ntff_profile_hook = set_axon_ntff_profile_hook
    mod.get_axon_ntff_profile_hook = get_axon_ntff_profile_hook
    sys.modules["antenv.axon_hooks"] = mod
    antenv.axon_hooks = mod
    try:
        from trn_agent_boot.trn_boot import _ntff_profile_via_ctypes

        hook = _ntff_profile_via_ctypes("/opt/axon/libaxon_pjrt.so")
        if hook is not None:
            mod._hook = hook
    except Exception:
        pass


_ensure_axon_hooks()

_NC_CACHE = None
LAST_RESULTS = None  # test harness reads .exec_time_ns off this


def _build_nc():
    import concourse.mybir as mybir
    import concourse.tile as tile
    from concourse import bacc

    f16 = mybir.dt.float16
    bf16 = mybir.dt.bfloat16
    f32 = mybir.dt.float32
    Exp = mybir.ActivationFunctionType.Exp

    nc = bacc.Bacc("TRN2", target_bir_lowering=False, debug=False, num_devices=NCORES)

    xt = nc.dram_tensor("xt", [D, S], f16, kind="ExternalInput").ap()
    yt = nc.dram_tensor("yt", [D, N], f16, kind="ExternalInput").ap()
    ytk = nc.dram_tensor("ytk", [D, S], f16, kind="ExternalInput").ap()
    out_rows = nc.dram_tensor("rowsums", [P, MT * NT], f32, kind="ExternalOutput").ap()
    out_cols = nc.dram_tensor("colsums", [4, NT * 512], f32, kind="ExternalOutput").ap()
    out_diag = nc.dram_tensor("diag", [4, 512], f32, kind="ExternalOutput").ap()

    # d-major layouts with the 256-long d axis split as (chunk c, partition p)
    xt_r = xt.rearrange("(c p) s -> p c s", p=P)
    yt_r = yt.rearrange("(c p) n -> p c n", p=P)
    ytk_r = ytk.rearrange("(c p) s -> p c s", p=P)

    with tile.TileContext(nc) as tc:
        with (
            tc.tile_pool(name="singles", bufs=1) as singles,
            tc.tile_pool(name="epool", bufs=3) as epool,
        ):
            # persistent SBUF residents
            xt_sb = singles.tile([P, 2, S], f16, tag="xt")
            ytk_sb = singles.tile([P, 2, S], f16, tag="ytk")
            # y-transpose pieces; the first two are small so the first
            # matmuls can start as early as possible
            piece_widths = [512, 1536] + [CHUNK] * (NT - 1)
            piece_starts = [0]
            for w in piece_widths:
                piece_starts.append(piece_starts[-1] + w)
            assert piece_starts[-1] == N
            yt_sb = [
                singles.tile([P, 2, w], f16, tag=f"yt{j}", name=f"yt{j}")
                for j, w in enumerate(piece_widths)
            ]

            def yt_slice(c, g0, width):
                """AP into the yt piece covering global cols [g0, g0+width)."""
                for j, s in enumerate(piece_starts[:-1]):
                    if s <= g0 and g0 + width <= piece_starts[j + 1]:
                        return yt_sb[j][:, c, g0 - s : g0 - s + width]
                raise AssertionError(f"no piece covers [{g0}, {g0 + width})")
            ones_f16 = singles.tile([P, 1], f16, tag="ones_f16")
            bias_sb = singles.tile([P, 1], f32, tag="bias")
            rowsum_all = singles.tile([P, MT * NT], f32, tag="rowsum")
            # colsum lives on partitions {0,32,64,96}: row 32q, col nt*512+r
            # holds colsum[nt*2048 + q*512 + r]
            colsum_sb = singles.tile([P, NT * 512], f32, tag="colsum")
            diag_sb = singles.tile([P, 512], f32, tag="diag")
            prod_sb = singles.tile([P, 2, S], f16, tag="prod")

            nc.vector.memset(ones_f16, 1.0)
            nc.vector.memset(bias_sb, -C_OFF)

            # One sync ring (the ACT-issued ring starts too slowly), ordered
            # by first use: main(0,0) needs yt piece 0 + only the first 128
            # xt columns (both K-halves); ytk (diag-only) is consumed last.
            nc.sync.dma_start(out=yt_sb[0], in_=yt_r[:, :, 0 : piece_widths[0]])
            nc.sync.dma_start(out=xt_sb[:, :, 0:P], in_=xt_r[:, :, 0:P])
            nc.sync.dma_start(out=xt_sb[:, :, P:S], in_=xt_r[:, :, P:S])
            for j in range(1, len(piece_widths)):
                s = piece_starts[j]
                nc.sync.dma_start(
                    out=yt_sb[j], in_=yt_r[:, :, s : s + piece_widths[j]]
                )
            nc.sync.dma_start(out=ytk_sb, in_=ytk_r)
            # elementwise x*y for the diagonal; runs early on the idle DVE,
            # consumed by the ones-matmuls after the main loop frees PSUM
            nc.vector.tensor_mul(prod_sb, xt_sb, ytk_sb)

            # ---- main loop over the 2048 x 16384 logits block ----
            # Per 2048-wide chunk: 16 row-tiles of matmul (fp16, 4x 512-wide
            # PSUM sub-tiles) + one wide exp on ScalarE (accum_out = row sums);
            # DVE accumulates the 16 bf16 E tiles (2x mode) then folds the
            # 128 partitions with a halving tree. PSUM: 2x 4-bank logits tiles.
            with (
                tc.tile_pool(name="pslogits", bufs=2, space="PSUM") as ps_pool,
                tc.tile_pool(name="accpool", bufs=NT) as accpool,
                tc.tile_pool(name="onespool", bufs=1) as onespool,
            ):
                ones_acc = onespool.tile([P, 1], bf16, tag="ones_acc")
                nc.vector.memset(ones_acc, 1.0)

                # All NT chunk accumulators stay resident; every colsum fold
                # happens after the last exp, so the steady-state loop has no
                # PSUM write-after-read stalls at chunk boundaries at all.
                accs = []
                prev_ps = last_ps = None
                for nt in range(NT):
                    acc = accpool.tile([P, CHUNK], bf16, tag="acc", name=f"acc{nt}")
                    accs.append(acc)
                    for mt in range(MT):
                        ps = ps_pool.tile([P, CHUNK], f32)
                        prev_ps, last_ps = last_ps, ps
                        for c in range(2):
                            for q in range(CHUNK // 512):
                                o = q * 512
                                nc.tensor.matmul(
                                    ps[:, o : o + 512],
                                    xt_sb[:, c, mt * P : (mt + 1) * P],
                                    yt_slice(c, nt * CHUNK + o, 512),
                                    start=(c == 0),
                                    stop=(c == 1),
                                )
                        e_sb = epool.tile([P, CHUNK], bf16, tag="e")
                        idx = mt * NT + nt
                        nc.scalar.activation(
                            out=e_sb,
                            in_=ps,
                            func=Exp,
                            bias=bias_sb[:],
                            scale=1.0,
                            accum_out=rowsum_all[:, idx : idx + 1],
                        )
                        if mt == 0:
                            nc.vector.tensor_copy(acc, e_sb)
                        else:
                            nc.vector.tensor_add(acc, acc, e_sb)

                # tail folds: chunks 0..3 into the second-to-last logits tile
                # (drained one exp earlier -> folds overlap the final exp),
                # chunks 4..7 into the last. Col-groups 0/32/64/96 x bank
                # (nt%4) give every fold a distinct PSUM cell.
                for nt in range(NT):
                    target = prev_ps if nt < NT // 2 else last_ps
                    off = (nt % (NT // 2)) * 512
                    for q in range(CHUNK // 512):
                        nc.tensor.matmul(
                            target[32 * q : 32 * q + 1, off : off + 512],
                            ones_acc,
                            accs[nt][:, q * 512 : (q + 1) * 512],
                            start=True,
                            stop=True,
                            tile_position=(0, 32 * q),
                        )
                nc.vector.tensor_copy(colsum_sb[0:97, 0:CHUNK], prev_ps[0:97, :])
                nc.vector.tensor_copy(
                    colsum_sb[0:97, CHUNK : 2 * CHUNK], last_ps[0:97, :]
                )
            nc.sync.dma_start(out=out_cols, in_=colsum_sb[0:97:32, 0 : 2 * CHUNK])

            # ---- diagonal: diag[s] = sum_d x[s,d]*y[s,d] (PSUM now free) ----
            # col-grouped ones-matmuls (concurrent) into one PSUM bank; rows
            # {0,32,64,96} x 512 hold diag[q*512 + r]
            with tc.tile_pool(name="psdiag", bufs=1, space="PSUM") as dg_pool:
                diag_ps = dg_pool.tile([P, 512], f32)
                for c in range(2):
                    for q in range(S // 512):
                        nc.tensor.matmul(
                            diag_ps[32 * q : 32 * q + 1, 0:512],
                            ones_f16,
                            prod_sb[:, c, q * 512 : (q + 1) * 512],
                            start=(c == 0),
                            stop=(c == 1),
                            tile_position=(0, 32 * q),
                        )
                nc.vector.tensor_copy(diag_sb[0:97, :], diag_ps[0:97, :])
            nc.sync.dma_start(out=out_diag, in_=diag_sb[0:97:32, :])

            nc.sync.dma_start(out=out_rows, in_=rowsum_all)

    nc.compile()
    return nc


def kernel(x, y):
    global _NC_CACHE, LAST_RESULTS
    from concourse.bass_utils import run_bass_kernel_spmd

    x = np.asarray(x, dtype=np.float32)
    y = np.asarray(y, dtype=np.float32)
    assert x.shape == (N, D) and y.shape == (N, D)

    xt_all = np.ascontiguousarray(x.T.astype(np.float16))  # [D, N]
    yt_all = np.ascontiguousarray(y.T.astype(np.float16))  # [D, N]

    in_maps = []
    for k in range(NCORES):
        sl = slice(k * S, (k + 1) * S)
        in_maps.append(
            {
                "xt": np.ascontiguousarray(xt_all[:, sl]),
                "yt": yt_all,
                "ytk": np.ascontiguousarray(yt_all[:, sl]),
            }
        )

    if _NC_CACHE is None:
        _NC_CACHE = _build_nc()
    nc = _NC_CACHE

    try:
        res = run_bass_kernel_spmd(nc, in_maps, core_ids=list(range(NCORES)))
    except Exception:
        # The tunneled device occasionally reports unrecoverable and resets
        # itself within a minute; one retry saves the run.
        import time

        time.sleep(75)
        res = run_bass_kernel_spmd(nc, in_maps, core_ids=list(range(NCORES)))
    LAST_RESULTS = res

    rowsum = np.empty(N, dtype=np.float64)
    colsum = np.zeros(N, dtype=np.float64)
    diag = np.empty(N, dtype=np.float64)
    for k in range(NCORES):
        r = res.results[k]
        # rowsums [P, MT*NT]: partition p, col mt*NT + nt -> row k*S + mt*P + p
        rs = r["rowsums"].astype(np.float64).reshape(P, MT, NT).sum(-1)  # [P, MT]
        rowsum[k * S : (k + 1) * S] = rs.T.reshape(S)
        # colsums [4, 2*CHUNK]: row q, col half*CHUNK + nt4*512 + r holds
        # colsum[(half*4 + nt4)*CHUNK + q*512 + r]
        cs = r["colsums"].astype(np.float64).reshape(4, 2, NT // 2, 512)
        colsum += np.transpose(cs, (1, 2, 0, 3)).reshape(N)
        diag[k * S : (k + 1) * S] = r["diag"].astype(np.float64).reshape(S)

    lse_row = C_OFF + np.log(rowsum)
    lse_col = C_OFF + np.log(colsum)
    loss = 0.5 * (lse_row.mean() + lse_col.mean()) - diag.mean()
    return np.float32(loss)


# revision 53
# speedup vs baseline: 1.0638x; 1.0068x over previous
"""CLIP loss (nn_CLIPLoss) on 8 Trainium2 NeuronCores.

loss = 0.5*(mean_i LSE_row(i) + mean_j LSE_col(j)) - mean_i <x_i, y_i>
with logits = x @ y.T, N=16384, D=256.

Strategy: shard x row-wise across 8 cores (2048 rows each); every core holds
all of y (host pre-transposes/casts both to fp16 [D, *] so the contraction dim
sits on SBUF partitions). Core k computes its 2048x16384 block of logits
tile-by-tile on the TensorEngine (fp16 operands, fp32 PSUM accumulate, 2048-
wide double-buffered PSUM tiles), applies exp(logit - C) on the ScalarEngine
(bf16 output for range, fp32 accum_out producing row-sums for free), and
accumulates the E tiles on VectorE (bf16 2x mode); the 128-partition column
fold is 4 col-group-concurrent ones-matmuls on the TensorEngine into an
already-drained logits PSUM tile. The diagonal <x_i, y_i> is an elementwise
product + ones-matmul on the same fp16 operands. The host sums the per-core
partials in float64, takes logs, and combines into the scalar.

The full NxN logits matrix is never materialized anywhere. Measured ~312 us
on hardware per 8-core run; ScalarE exp throughput (~266 us) is the floor.
"""

import os
import sys
import types

import numpy as np

N = 16384
D = 256
NCORES = 8
S = N // NCORES  # 2048 rows per core
P = 128
C_OFF = 64.0  # exp offset: logits in ~[-100, 100]; exp(l - 64) stays in fp32/bf16 range
NT = 8  # 2048-wide column chunks
CHUNK = 2048
MT = S // P  # 16 row tiles per core
YT_PIECES = 8  # split the big y-transpose load into 8 DMAs for pipelining


def _ensure_axon_hooks():
    """The agent image's antenv lacks axon_hooks; recreate it and register the
    ctypes NTFF profile hook so BASS_TRACE=1 yields exec_time_ns. Harmless
    no-op when profiling infra is absent."""
    try:
        import antenv.axon_hooks  # noqa: F401

        return
    except ImportError:
        pass
    try:
        import antenv
    except ImportError:
        return

    mod = types.ModuleType("antenv.axon_hooks")
    mod._hook = None

    def set_axon_ntff_profile_hook(h):
        mod._hook = h

    def get_axon_ntff_profile_hook():
        return mod._hook

    mod.set_axon_ntff_profile_hook = set_axon_ntff_profile_hook
    mod.get_axon_ntff_profile_hook = get_axon_ntff_profile_hook
    sys.modules["antenv.axon_hooks"] = mod
    antenv.axon_hooks = mod
    try:
        from trn_agent_boot.trn_boot import _ntff_profile_via_ctypes

        hook = _ntff_profile_via_ctypes("/opt/axon/libaxon_pjrt.so")
        if hook is not None:
            mod._hook = hook
    except Exception:
        pass


_ensure_axon_hooks()

_NC_CACHE = None
LAST_RESULTS = None  # test harness reads .exec_time_ns off this


def _build_nc():
    import concourse.mybir as mybir
    import concourse.tile as tile
    from concourse import bacc

    f16 = mybir.dt.float16
    bf16 = mybir.dt.bfloat16
    f32 = mybir.dt.float32
    Exp = mybir.ActivationFunctionType.Exp

    nc = bacc.Bacc("TRN2", target_bir_lowering=False, debug=False, num_devices=NCORES)

    xt = nc.dram_tensor("xt", [D, S], f16, kind="ExternalInput").ap()
    yt = nc.dram_tensor("yt", [D, N], f16, kind="ExternalInput").ap()
    ytk = nc.dram_tensor("ytk", [D, S], f16, kind="ExternalInput").ap()
    out_rows = nc.dram_tensor("rowsums", [P, MT * NT], f32, kind="ExternalOutput").ap()
    out_cols = nc.dram_tensor("colsums", [4, NT * 512], f32, kind="ExternalOutput").ap()
    out_diag = nc.dram_tensor("diag", [4, 512], f32, kind="ExternalOutput").ap()

    # d-major layouts with the 256-long d axis split as (chunk c, partition p)
    xt_r = xt.rearrange("(c p) s -> p c s", p=P)
    yt_r = yt.rearrange("(c p) n -> p c n", p=P)
    ytk_r = ytk.rearrange("(c p) s -> p c s", p=P)

    with tile.TileContext(nc) as tc:
        with (
            tc.tile_pool(name="singles", bufs=1) as singles,
            tc.tile_pool(name="epool", bufs=3) as epool,
        ):
            # persistent SBUF residents
            xt_sb = singles.tile([P, 2, S], f16, tag="xt")
            ytk_sb = singles.tile([P, 2, S], f16, tag="ytk")
            # y-transpose pieces; the first two are small so the first
            # matmuls can start as early as possible
            piece_widths = [512, 1536] + [CHUNK] * (NT - 1)
            piece_starts = [0]
            for w in piece_widths:
                piece_starts.append(piece_starts[-1] + w)
            assert piece_starts[-1] == N
            yt_sb = [
                singles.tile([P, 2, w], f16, tag=f"yt{j}", name=f"yt{j}")
                for j, w in enumerate(piece_widths)
            ]

            def yt_slice(c, g0, width):
                """AP into the yt piece covering global cols [g0, g0+width)."""
                for j, s in enumerate(piece_starts[:-1]):
                    if s <= g0 and g0 + width <= piece_starts[j + 1]:
                        return yt_sb[j][:, c, g0 - s : g0 - s + width]
                raise AssertionError(f"no piece covers [{g0}, {g0 + width})")
            ones_f16 = singles.tile([P, 1], f16, tag="ones_f16")
            bias_sb = singles.tile([P, 1], f32, tag="bias")
            rowsum_all = singles.tile([P, MT * NT], f32, tag="rowsum")
            # colsum lives on partitions {0,32,64,96}: row 32q, col nt*512+r
            # holds colsum[nt*2048 + q*512 + r]
            colsum_sb = singles.tile([P, NT * 512], f32, tag="colsum")
            diag_sb = singles.tile([P, 512], f32, tag="diag")
            prod_sb = singles.tile([P, 2, S], f16, tag="prod")

            nc.vector.memset(ones_f16, 1.0)
            nc.vector.memset(bias_sb, -C_OFF)

            # One sync ring (the ACT-issued ring starts too slowly), ordered
            # by first use: main(0,0) needs yt piece 0 + only the first 128
            # xt columns (both K-halves); ytk (diag-only) is consumed last.
            nc.sync.dma_start(out=yt_sb[0], in_=yt_r[:, :, 0 : piece_widths[0]])
            nc.sync.dma_start(out=xt_sb[:, :, 0:P], in_=xt_r[:, :, 0:P])
            nc.sync.dma_start(out=xt_sb[:, :, P:S], in_=xt_r[:, :, P:S])
            for j in range(1, len(piece_widths)):
                s = piece_starts[j]
                nc.sync.dma_start(
                    out=yt_sb[j], in_=yt_r[:, :, s : s + piece_widths[j]]
                )
            nc.sync.dma_start(out=ytk_sb, in_=ytk_r)
            # elementwise x*y for the diagonal; runs early on the idle DVE,
            # consumed by the ones-matmuls after the main loop frees PSUM
            nc.vector.tensor_mul(prod_sb, xt_sb, ytk_sb)

            # ---- main loop over the 2048 x 16384 logits block ----
            # Per 2048-wide chunk: 16 row-tiles of matmul (fp16, 4x 512-wide
            # PSUM sub-tiles) + one wide exp on ScalarE (accum_out = row sums);
            # DVE accumulates the 16 bf16 E tiles (2x mode) then folds the
            # 128 partitions with a halving tree. PSUM: 2x 4-bank logits tiles.
            with (
                tc.tile_pool(name="pslogits", bufs=2, space="PSUM") as ps_pool,
                tc.tile_pool(name="accpool", bufs=NT) as accpool,
                tc.tile_pool(name="onespool", bufs=1) as onespool,
            ):
                ones_acc = onespool.tile([P, 1], bf16, tag="ones_acc")
                nc.vector.memset(ones_acc, 1.0)

                # All NT chunk accumulators stay resident; every colsum fold
                # happens after the last exp, so the steady-state loop has no
                # PSUM write-after-read stalls at chunk boundaries at all.
                accs = []
                prev_ps = last_ps = None
                for nt in range(NT):
                    acc = accpool.tile([P, CHUNK], bf16, tag="acc", name=f"acc{nt}")
                    accs.append(acc)
                    for mt in range(MT):
                        ps = ps_pool.tile([P, CHUNK], f32)
                        prev_ps, last_ps = last_ps, ps
                        for c in range(2):
                            for q in range(CHUNK // 512):
                                o = q * 512
                                nc.tensor.matmul(
                                    ps[:, o : o + 512],
                                    xt_sb[:, c, mt * P : (mt + 1) * P],
                                    yt_slice(c, nt * CHUNK + o, 512),
                                    start=(c == 0),
                                    stop=(c == 1),
                                )
                        e_sb = epool.tile([P, CHUNK], bf16, tag="e")
                        idx = mt * NT + nt
                        nc.scalar.activation(
                            out=e_sb,
                            in_=ps,
                            func=Exp,
                            bias=bias_sb[:],
                            scale=1.0,
                            accum_out=rowsum_all[:, idx : idx + 1],
                        )
                        if mt == 0:
                            nc.vector.tensor_copy(acc, e_sb)
                        else:
                            nc.vector.tensor_add(acc, acc, e_sb)

                # tail folds: chunks 0..3 into the second-to-last logits tile
                # (drained one exp earlier -> folds overlap the final exp),
                # chunks 4..7 into the last. Col-groups 0/32/64/96 x bank
                # (nt%4) give every fold a distinct PSUM cell.
                def fold(nt):
                    target = prev_ps if nt < NT // 2 else last_ps
                    off = (nt % (NT // 2)) * 512
                    for q in range(CHUNK // 512):
                        nc.tensor.matmul(
                            target[32 * q : 32 * q + 1, off : off + 512],
                            ones_acc,
                            accs[nt][:, q * 512 : (q + 1) * 512],
                            start=True,
                            stop=True,
                            tile_position=(0, 32 * q),
                        )

                for nt in range(NT // 2):
                    fold(nt)
                # this copy overlaps the final exp on ScalarE
                nc.vector.tensor_copy(colsum_sb[0:97, 0:CHUNK], prev_ps[0:97, :])
                for nt in range(NT // 2, NT):
                    fold(nt)
                nc.vector.tensor_copy(
                    colsum_sb[0:97, CHUNK : 2 * CHUNK], last_ps[0:97, :]
                )
            nc.sync.dma_start(out=out_cols, in_=colsum_sb[0:97:32, 0 : 2 * CHUNK])

            # ---- diagonal: diag[s] = sum_d x[s,d]*y[s,d] (PSUM now free) ----
            # col-grouped ones-matmuls (concurrent) into one PSUM bank; rows
            # {0,32,64,96} x 512 hold diag[q*512 + r]
            with tc.tile_pool(name="psdiag", bufs=1, space="PSUM") as dg_pool:
                diag_ps = dg_pool.tile([P, 512], f32)
                for c in range(2):
                    for q in range(S // 512):
                        nc.tensor.matmul(
                            diag_ps[32 * q : 32 * q + 1, 0:512],
                            ones_f16,
                            prod_sb[:, c, q * 512 : (q + 1) * 512],
                            start=(c == 0),
                            stop=(c == 1),
                            tile_position=(0, 32 * q),
                        )
                nc.vector.tensor_copy(diag_sb[0:97, :], diag_ps[0:97, :])
            nc.sync.dma_start(out=out_diag, in_=diag_sb[0:97:32, :])

            nc.sync.dma_start(out=out_rows, in_=rowsum_all)

    nc.compile()
    return nc


def kernel(x, y):
    global _NC_CACHE, LAST_RESULTS
    from concourse.bass_utils import run_bass_kernel_spmd

    x = np.asarray(x, dtype=np.float32)
    y = np.asarray(y, dtype=np.float32)
    assert x.shape == (N, D) and y.shape == (N, D)

    xt_all = np.ascontiguousarray(x.T.astype(np.float16))  # [D, N]
    yt_all = np.ascontiguousarray(y.T.astype(np.float16))  # [D, N]

    in_maps = []
    for k in range(NCORES):
        sl = slice(k * S, (k + 1) * S)
        in_maps.append(
            {
                "xt": np.ascontiguousarray(xt_all[:, sl]),
                "yt": yt_all,
                "ytk": np.ascontiguousarray(yt_all[:, sl]),
            }
        )

    if _NC_CACHE is None:
        _NC_CACHE = _build_nc()
    nc = _NC_CACHE

    try:
        res = run_bass_kernel_spmd(nc, in_maps, core_ids=list(range(NCORES)))
    except Exception:
        # The tunneled device occasionally reports unrecoverable and resets
        # itself within a minute; one retry saves the run.
        import time

        time.sleep(75)
        res = run_bass_kernel_spmd(nc, in_maps, core_ids=list(range(NCORES)))
    LAST_RESULTS = res

    rowsum = np.empty(N, dtype=np.float64)
    colsum = np.zeros(N, dtype=np.float64)
    diag = np.empty(N, dtype=np.float64)
    for k in range(NCORES):
        r = res.results[k]
        # rowsums [P, MT*NT]: partition p, col mt*NT + nt -> row k*S + mt*P + p
        rs = r["rowsums"].astype(np.float64).reshape(P, MT, NT).sum(-1)  # [P, MT]
        rowsum[k * S : (k + 1) * S] = rs.T.reshape(S)
        # colsums [4, 2*CHUNK]: row q, col half*CHUNK + nt4*512 + r holds
        # colsum[(half*4 + nt4)*CHUNK + q*512 + r]
        cs = r["colsums"].astype(np.float64).reshape(4, 2, NT // 2, 512)
        colsum += np.transpose(cs, (1, 2, 0, 3)).reshape(N)
        diag[k * S : (k + 1) * S] = r["diag"].astype(np.float64).reshape(S)

    lse_row = C_OFF + np.log(rowsum)
    lse_col = C_OFF + np.log(colsum)
    loss = 0.5 * (lse_row.mean() + lse_col.mean()) - diag.mean()
    return np.float32(loss)
